# revision 12
# baseline (speedup 1.0000x reference)
"""Trainium2 Bass kernel for causal multi-head attention (B=4, T=2048, D=1024, H=16).

Sharding: 8 cores = 4 batches x 2 head-groups (8 heads each), no collectives:
each core emits a full-[1024, T] bf16 partial of the output projection and the
host sums the pair per batch (the TP "unshard").

Per core (Tile framework, single SPMD program):
  proj(j):  Q/K/V projections in fp8e4 DoubleRow (2 fp8 MACs/cell/cycle,
            contraction 1024 = 4 x [128,2] chunks).  Weights carry power-of-2
            scales (wq 2^12 incl 1/sqrt(hd), wk 2^6, wv 2^6) so fp8 values sit
            in e4m3's sweet spot; the exp activation's scale compensates
            (2^-18) and the V ones-column is 64.0 (cancels in normalize).
  attn(j,p): causal flash attention in transposed layout, bf16 (HD=64 gives
            no fp8 win: DoubleRow needs contraction >= 256).  ST[k,q] =
            KT^T @ QT (head pair packed via tile_position), additive mask on
            the diagonal slab (DVE, pre-exp), PT = exp(ST * 2^-18) (ACT),
            O[hd+1, q] += [V|64]^T @ PT.
  normalize: rowsum row [1,1024] (from the V ones-column) -> one SBUF->SBUF
            transposing DMA -> [128,8] -> DVE reciprocal (parallel lanes) ->
            bf16 cast -> transposing DMA back to row form -> PE outer-product
            broadcast (ones[1,64] x recrow) into PSUM -> 2 DVE muls.
            The chain is lag-1 pipelined: pair p's reciprocal+return-hop are
            emitted after pair p+1's first block (DVE is idle there: the only
            DVE work early in a pair is nothing - mask adds are diagonal-only
            and proj casts moved to ACT), and the broadcast+muls at pair
            p+1's end (by which time the DMA hops have landed).
  outproj(j): YT[dout, q] = Wo_g^T @ OT + bias (g=0 adds bo), bf16.

Scheduling: proj/outproj work is chopped into single-instruction generator
fillers drained into the attention blocks' tensor bubbles (exp at 1 elem/
lane/cycle outpaces the 2+2 matmuls per block by ~0.25us).  proj(j+1) drains
during attn(j); outproj(j) drains one j later (its ot inputs are written by
the lag-1 muls).  PSUM->SBUF proj casts run on the ACT engine so the tensor
engine's resume at j-boundaries never sits behind normalize work on DVE.

hw quirks inherited from the baseline: custom-DVE ops silently corrupt on
this runtime; gpsimd is slow (~1us/op) except memset; engine partition
offsets must be multiples of 32; DVE ops allow per-operand partition BASE
offsets (sizes must match); matmul PSUM output must stay within one 2KB
bank (N <= 512 fp32).

Host: transpose/slice/scale weights to fp8/bf16, sum per-core partials,
assemble [B, T, D].
"""

import numpy as np

B, T, D, H, HD = 4, 2048, 1024, 16, 64
NCORES = 8
NP = 4          # head pairs per core
NJ = 4          # q-ranges of 512
QW = 512
TB = T // 128   # 16

# power-of-2 scale folding for the fp8 Q/K projections
SQ_EXP = 12     # wq scale 2^12 (on top of 1/sqrt(HD))
SK_EXP = 6      # wk scale 2^6
SV = 1.0        # V path is bf16: no scale; ones column stays 1.0
EXP_SCALE = 2.0 ** (-(SQ_EXP + SK_EXP))
MASK_VAL = -50.0 * 2.0 ** (SQ_EXP + SK_EXP)

_CACHE = {}


def _build_nc():
    import concourse.mybir as mybir
    import concourse.tile as tile
    from concourse import bacc

    F32 = mybir.dt.float32
    BF16 = mybir.dt.bfloat16
    FP8 = mybir.dt.float8e4
    AF = mybir.ActivationFunctionType
    DR = mybir.MatmulPerfMode.DoubleRow

    nc = bacc.Bacc(None, target_bir_lowering=False)
    xt_d = nc.declare_dram_parameter("xt", [D, T], FP8, isOutput=False)
    xtv_d = nc.declare_dram_parameter("xtv", [D, T], BF16, isOutput=False)
    wq_d = nc.declare_dram_parameter("wq", [D, 512], FP8, isOutput=False)
    wk_d = nc.declare_dram_parameter("wk", [D, 512], FP8, isOutput=False)
    wv_d = nc.declare_dram_parameter("wv", [D, 512], BF16, isOutput=False)
    wo_d = nc.declare_dram_parameter("wo", [512, D], BF16, isOutput=False)
    bias_d = nc.declare_dram_parameter("bias", [128, 8], F32, isOutput=False)
    mask_d = nc.declare_dram_parameter("mask", [128, 256], BF16, isOutput=False)
    yt_d = nc.declare_dram_parameter("yt", [D, T], BF16, isOutput=True)

    with tile.TileContext(nc) as tc:
        with (
            tc.tile_pool(name="persist", bufs=1) as pers,
            tc.tile_pool(name="work", bufs=1) as work,
            tc.tile_pool(name="dram", bufs=1, space="DRAM") as dram,
            tc.tile_pool(name="psum", bufs=1, space="PSUM") as psum,
        ):
            rsum_d = dram.tile([NP, 1024], F32)
            rec_d = dram.tile([NP, 1024], mybir.dt.float16)
            F16 = mybir.dt.float16
            rsT8 = pers.tile([128, NP, 8], F32)
            rec8 = pers.tile([128, NP, 8], F32)
            rec8b = pers.tile([128, NP, 8], F16)
            recrow = pers.tile([1, NP * 1024], F16)
            ones64 = pers.tile([1, 64], F16)
            qt = pers.tile([128, NP, T], BF16)
            kt = pers.tile([128, NP, T], BF16)
            v = pers.tile([128, TB, 8 * 65], BF16)
            ot = pers.tile([128, NP, T], BF16)
            mneg = pers.tile([128, 256], BF16)
            wo = pers.tile([128, 4, D], BF16)
            bias = pers.tile([128, 8], F32)
            wq = pers.tile([128, 8, 512], FP8)
            wk = pers.tile([128, 8, 512], FP8)
            wv = pers.tile([128, 8, 512], BF16)
            xsb = pers.tile([128, 8, T], FP8)
            xv = pers.tile([128, 8, T], BF16)

            # DMA priority order: operands of the first matmuls first.
            for c in range(8):
                nc.sync.dma_start(
                    out=xsb[:, c, 0:QW], in_=xt_d[c * 128:(c + 1) * 128, 0:QW]
                )
                nc.sync.dma_start(out=wq[:, c, :], in_=wq_d[c * 128:(c + 1) * 128, :])
            nc.sync.dma_start(out=mneg[:], in_=mask_d[:])
            nc.sync.dma_start(out=bias[:], in_=bias_d[:])
            for c in range(8):
                nc.sync.dma_start(out=wk[:, c, :], in_=wk_d[c * 128:(c + 1) * 128, :])
            for c in range(8):
                nc.sync.dma_start(
                    out=xv[:, c, 0:QW], in_=xtv_d[c * 128:(c + 1) * 128, 0:QW]
                )
                nc.sync.dma_start(out=wv[:, c, :], in_=wv_d[c * 128:(c + 1) * 128, :])
            for c in range(8):
                nc.sync.dma_start(
                    out=xsb[:, c, QW:T], in_=xt_d[c * 128:(c + 1) * 128, QW:T]
                )
            for c in range(8):
                nc.sync.dma_start(
                    out=xv[:, c, QW:T], in_=xtv_d[c * 128:(c + 1) * 128, QW:T]
                )
            for c in range(4):
                nc.sync.dma_start(out=wo[:, c, :], in_=wo_d[c * 128:(c + 1) * 128, :])
            nc.vector.memset(ones64[:], 1.0)
            # the V ones-columns never change: set once (SV cancels in normalize)
            nc.vector.memset(
                v[:].rearrange("p t (h c) -> p t h c", h=8)[:, :, :, 64:65], SV
            )
            # pre-warm the Exp activation table
            nc.scalar.activation(rsT8[0:1, 0, 0:1], bias[0:1, 0:1], AF.Exp)

            # Filler units: projection / output-projection work chopped into
            # single-instruction generator steps, drained one step at a time
            # into the tensor-idle bubbles of the attention block loop.
            def q_gen(j, p, w_sb=wq, dst=qt):
                jr = slice(j * QW, (j + 1) * QW)
                acc = psum.tile([128, QW], F32, tag="small", bufs=2)
                for cc in range(4):
                    nc.tensor.matmul(
                        acc[:],
                        w_sb[:, 2 * cc:2 * cc + 2, p * 128:(p + 1) * 128],
                        xsb[:, 2 * cc:2 * cc + 2, jr],
                        start=(cc == 0),
                        stop=(cc == 3),
                        perf_mode=DR,
                    )
                    yield
                nc.scalar.copy(dst[:, p, jr], acc[:])
                yield

            def k_gen(j, p):
                yield from q_gen(j, p, w_sb=wk, dst=kt)

            def v_gen(j, sub):
                i = 4 * j + sub
                ir = slice(i * 128, (i + 1) * 128)
                acc = psum.tile([128, QW], F32, tag="small", bufs=2)
                for c in range(8):
                    nc.tensor.matmul(
                        acc[:],
                        xv[:, c, ir],
                        wv[:, c, :],
                        start=(c == 0),
                        stop=(c == 7),
                    )
                    yield
                vblk = v[:, i, :].rearrange("p (h c) -> p h c", c=65)
                nc.scalar.copy(
                    vblk[:, :, 0:64],
                    acc[:].rearrange("p (h c) -> p h c", c=64),
                )
                yield

            def outproj_gen(j):
                jr = slice(j * QW, (j + 1) * QW)
                for n in range(8):
                    yps = psum.tile([128, QW], F32, tag="small", bufs=2)
                    for c in range(4):
                        nc.tensor.matmul(
                            yps[:],
                            wo[:, c, n * 128:(n + 1) * 128],
                            ot[:, c, jr],
                            start=(c == 0), stop=(c == 3),
                        )
                        yield
                    ysb = work.tile([128, QW], BF16, tag="ysb", bufs=3)
                    nc.vector.tensor_scalar_add(ysb[:], yps[:], bias[:, n:n + 1])
                    nc.sync.dma_start(
                        out=yt_d[n * 128:(n + 1) * 128, jr], in_=ysb[:]
                    )
                    yield

            pending = []

            def drain(n):
                while n > 0 and pending:
                    try:
                        next(pending[0])
                        n -= 1
                    except StopIteration:
                        pending.pop(0)

            def drain_all():
                while pending:
                    try:
                        next(pending[0])
                    except StopIteration:
                        pending.pop(0)

            def proj0():
                pending.extend([q_gen(0, p) for p in range(NP)])
                pending.extend([k_gen(0, p) for p in range(NP)])
                pending.extend([v_gen(0, sub) for sub in range(4)])
                drain_all()

            # --- normalize chain pieces (lag-1 pipelined) ---------------- #
            def norm_start(j, p, ocp):
                """rowsum row -> [128,8]: bounce through DRAM (linear DRAM
                APs absorb the transpose; SBUF<->SBUF cannot balance it)."""
                nc.sync.dma_start(out=rsum_d[p:p + 1, :], in_=ocp[64:65, :])
                nc.sync.dma_start(
                    out=rsT8[:, p, :],
                    in_=rsum_d[p:p + 1, :].rearrange("p (c q) -> p q c", c=8),
                )

            def norm_recip(j, p):
                """reciprocal in transposed form + return transposing hops."""
                nc.vector.reciprocal(rec8[:, p, :], rsT8[:, p, :])
                nc.vector.tensor_copy(rec8b[:, p, :], rec8[:, p, :])
                nc.sync.dma_start(out=rec_d[p:p + 1, :], in_=rec8b[:, p, :])
                nc.sync.dma_start(
                    out=recrow[0:1, p * 1024:(p + 1) * 1024],
                    in_=rec_d[p:p + 1, :].rearrange("p (q c) -> p c q", c=8),
                )

            def norm_finish(j, p, ocp):
                """PE outer-product broadcast + the 2 normalize muls."""
                jr = slice(j * QW, (j + 1) * QW)
                bc = psum.tile([128, 1024], F32, tag="st", bufs=2)
                for s in range(2):
                    nc.tensor.matmul(
                        bc[s * 64:(s + 1) * 64, s * QW:(s + 1) * QW],
                        ones64[:],
                        recrow[0:1, p * 1024 + s * QW:p * 1024 + (s + 1) * QW],
                        start=True, stop=True,
                    )
                for s in range(2):
                    nc.vector.tensor_mul(
                        ot[s * 64:(s + 1) * 64, p, jr],
                        ocp[0:64, s * QW:(s + 1) * QW],
                        bc[s * 64:(s + 1) * 64, s * QW:(s + 1) * QW],
                    )

            # lag-1 state: (j, p, ocp, stage) of the previous pair
            prev = [None]

            def attn_p(j, p, drain_n):
                jr = slice(j * QW, (j + 1) * QW)
                hA, hB = 2 * p, 2 * p + 1
                o_A = psum.tile([65, QW], F32, tag="o", bufs=2)
                o_B = psum.tile([65, QW], F32, tag="o", bufs=2)
                nkb = 4 * j + 4
                for kb in range(nkb):
                    o = kb - 4 * j  # diagonal offset; < 0 means full block
                    lo = 128 * o if o > 0 else 0  # first live q col in range
                    st = psum.tile([128, 1024], F32, tag="st", bufs=2)
                    kcols = slice(kb * 128, (kb + 1) * 128)
                    qcols = slice(j * QW + lo, (j + 1) * QW)
                    nc.tensor.matmul(
                        st[:, lo:QW],
                        kt[0:64, p, kcols],
                        qt[0:64, p, qcols],
                        start=True, stop=True, tile_position=(0, 0),
                    )
                    nc.tensor.matmul(
                        st[:, QW + lo:2 * QW],
                        kt[64:128, p, kcols],
                        qt[64:128, p, qcols],
                        start=True, stop=True, tile_position=(64, 0),
                    )
                    stv = st[:].rearrange("p (h q) -> p h q", h=2)
                    if o >= 0:
                        # additive causal mask on the diagonal 128-slab
                        nc.vector.tensor_add(
                            stv[:, :, lo:lo + 128],
                            stv[:, :, lo:lo + 128],
                            mneg[:].rearrange("p (h q) -> p h q", h=2),
                        )
                    pt = work.tile([128, 1024], BF16, tag="pt", bufs=3)
                    nc.scalar.activation(
                        pt[:].rearrange("p (h q) -> p h q", h=2)[:, :, lo:QW],
                        stv[:, :, lo:QW],
                        AF.Exp,
                        scale=EXP_SCALE,
                    )
                    drain(drain_n)  # fill the exp-wait bubble with fillers
                    nc.tensor.matmul(
                        o_A[:, lo:QW],
                        v[:, kb, hA * 65:(hA + 1) * 65],
                        pt[:, lo:QW],
                        start=(kb == 0), stop=(kb == nkb - 1),
                    )
                    nc.tensor.matmul(
                        o_B[:, lo:QW],
                        v[:, kb, hB * 65:(hB + 1) * 65],
                        pt[:, QW + lo:2 * QW],
                        start=(kb == 0), stop=(kb == nkb - 1),
                    )
                    if kb == 0 and prev[0] is not None:
                        # previous pair's reciprocal: DVE is idle early in a
                        # pair (mask adds are diagonal-only), so the wait on
                        # its transpose-DMA hop is harmless here.
                        pj, pp, pocp = prev[0]
                        norm_recip(pj, pp)
                # copy psum accumulators out so the o slots free early;
                # DVE (not ACT: ACT paces the exp stream).
                ocp = work.tile([65, 1024], F32, tag="ocp", bufs=4)
                nc.vector.tensor_copy(ocp[:, 0:QW], o_A[:])
                nc.vector.tensor_copy(ocp[:, QW:1024], o_B[:])
                norm_start(j, p, ocp)
                if prev[0] is not None:
                    pj, pp, pocp = prev[0]
                    norm_finish(pj, pp, pocp)
                prev[0] = (j, p, ocp)

            proj0()
            # Filler supply per attention window: proj(j+1) is queued during
            # attn(j) and must fully drain before attn(j+1) (in-order tensor
            # queue).  outproj(j) is queued one pair into attn(j+1) so its ot
            # reads are emitted after the lag-1 normalize muls that write ot.
            DRAIN_N = {0: 3, 1: 2, 2: 3, 3: 2}
            for j in range(NJ):
                for p in range(NP):
                    if j + 1 < NJ:
                        pending.append(q_gen(j + 1, p))
                        pending.append(k_gen(j + 1, p))
                        if p == NP - 1:
                            pending.extend([v_gen(j + 1, sub) for sub in range(4)])
                    if j > 0 and p == 1:
                        pending.append(outproj_gen(j - 1))
                    attn_p(j, p, DRAIN_N[j])
                drain_all()
            # tail: finish the last pair inline, then the last outproj
            pj, pp, pocp = prev[0]
            norm_recip(pj, pp)
            norm_finish(pj, pp, pocp)
            pending.append(outproj_gen(3))
            drain_all()

    nc.finalize()
    return nc


def _prep_inputs(x, Wq, Wk, Wv, Wo, bo):
    """Build the 8 per-core input maps (host-side layout prep only)."""
    import ml_dtypes

    FP8 = ml_dtypes.float8_e4m3
    scale = np.float32(1.0 / np.sqrt(np.float32(HD)) * 2.0 ** SQ_EXP)
    kr = np.arange(128, dtype=np.float32)[:, None]
    qc = np.arange(128, dtype=np.float32)[None, :]
    tri = np.where(qc >= kr, np.float32(0.0), np.float32(MASK_VAL))
    mneg = np.tile(tri, (1, 2)).astype(ml_dtypes.bfloat16)

    in_maps = []
    for c in range(NCORES):
        b, g = c // 2, c % 2
        hs = slice(g * 8, (g + 1) * 8)
        xtf = np.ascontiguousarray(x[b].T)
        xt = xtf.astype(FP8)
        xtv = xtf.astype(ml_dtypes.bfloat16)
        wq = np.ascontiguousarray(Wq[hs].reshape(512, D).T * scale).astype(FP8)
        wk = np.ascontiguousarray(Wk[hs].reshape(512, D).T * np.float32(2.0 ** SK_EXP)).astype(FP8)
        wv = np.ascontiguousarray(Wv[hs].reshape(512, D).T).astype(ml_dtypes.bfloat16)
        wo = np.ascontiguousarray(Wo[:, g * 512:(g + 1) * 512].T).astype(ml_dtypes.bfloat16)
        if g == 0:
            bias = np.ascontiguousarray(bo.reshape(8, 128).T)
        else:
            bias = np.zeros((128, 8), np.float32)
        in_maps.append(
            {"xt": xt, "xtv": xtv, "wq": wq, "wk": wk, "wv": wv, "wo": wo,
             "bias": bias, "mask": mneg}
        )
    return in_maps


def _run(inputs, trace=False, trace_cores=None):
    from concourse.bass_utils import run_bass_kernel_spmd

    if "nc" not in _CACHE:
        _CACHE["nc"] = _build_nc()
    nc = _CACHE["nc"]
    in_maps = _prep_inputs(
        inputs["x"], inputs["Wq"], inputs["Wk"], inputs["Wv"], inputs["Wo"], inputs["bo"]
    )
    r = run_bass_kernel_spmd(
        nc, in_maps, list(range(NCORES)), trace=trace, trace_cores=trace_cores
    )
    y = np.empty((B, T, D), np.float32)
    for b in range(B):
        yt = np.asarray(r.results[2 * b]["yt"], dtype=np.float32) + np.asarray(
            r.results[2 * b + 1]["yt"], dtype=np.float32
        )
        y[b] = yt.T
    return y, r


def kernel(**inputs):
    y, _ = _run(inputs, trace=False)
    return y


# revision 16
# speedup vs baseline: 1.0906x; 1.0906x over previous
"""Trainium2 Bass kernel for causal multi-head attention (B=4, T=2048, D=1024, H=16).

Sharding: 8 cores = 4 batches x 2 head-groups (8 heads each), no collectives:
each core emits a full-[1024, T] bf16 partial of the output projection and the
host sums the pair per batch (the TP "unshard").

Per core (Tile framework, single SPMD program):
  proj(j):  Q/K/V projections in fp8e4 DoubleRow (2 fp8 MACs/cell/cycle,
            contraction 1024 = 4 x [128,2] chunks).  Weights carry power-of-2
            scales (wq 2^12 incl 1/sqrt(hd), wk 2^6, wv 2^6) so fp8 values sit
            in e4m3's sweet spot; the exp activation's scale compensates
            (2^-18) and the V ones-column is 64.0 (cancels in normalize).
  attn(j,p): causal flash attention in transposed layout, bf16 (HD=64 gives
            no fp8 win: DoubleRow needs contraction >= 256).  ST[k,q] =
            KT^T @ QT (head pair packed via tile_position), additive mask on
            the diagonal slab (DVE, pre-exp), PT = exp(ST * 2^-18) (ACT),
            O[hd+1, q] += [V|64]^T @ PT.
  normalize: rowsum row [1,1024] (from the V ones-column) -> one SBUF->SBUF
            transposing DMA -> [128,8] -> DVE reciprocal (parallel lanes) ->
            bf16 cast -> transposing DMA back to row form -> PE outer-product
            broadcast (ones[1,64] x recrow) into PSUM -> 2 DVE muls.
            The chain is lag-1 pipelined: pair p's reciprocal+return-hop are
            emitted after pair p+1's first block (DVE is idle there: the only
            DVE work early in a pair is nothing - mask adds are diagonal-only
            and proj casts moved to ACT), and the broadcast+muls at pair
            p+1's end (by which time the DMA hops have landed).
  outproj(j): YT[dout, q] = Wo_g^T @ OT + bias (g=0 adds bo), bf16.

Scheduling: proj/outproj work is chopped into single-instruction generator
fillers drained into the attention blocks' tensor bubbles (exp at 1 elem/
lane/cycle outpaces the 2+2 matmuls per block by ~0.25us).  proj(j+1) drains
during attn(j); outproj(j) drains one j later (its ot inputs are written by
the lag-1 muls).  PSUM->SBUF proj casts run on the ACT engine so the tensor
engine's resume at j-boundaries never sits behind normalize work on DVE.

hw quirks inherited from the baseline: custom-DVE ops silently corrupt on
this runtime; gpsimd is slow (~1us/op) except memset; engine partition
offsets must be multiples of 32; DVE ops allow per-operand partition BASE
offsets (sizes must match); matmul PSUM output must stay within one 2KB
bank (N <= 512 fp32).

Host: transpose/slice/scale weights to fp8/bf16, sum per-core partials,
assemble [B, T, D].
"""

import numpy as np

B, T, D, H, HD = 4, 2048, 1024, 16, 64
NCORES = 8
NP = 4          # head pairs per core
NJ = 4          # q-ranges of 512
QW = 512
TB = T // 128   # 16

# power-of-2 scale folding for the fp8 Q/K projections
SQ_EXP = 12     # wq scale 2^12 (on top of 1/sqrt(HD))
SK_EXP = 6      # wk scale 2^6
SV = 1.0        # V path is bf16: no scale; ones column stays 1.0
EXP_SCALE = 2.0 ** (-(SQ_EXP + SK_EXP))
MASK_VAL = -50.0 * 2.0 ** (SQ_EXP + SK_EXP)

_CACHE = {}


def _build_nc():
    import concourse.mybir as mybir
    import concourse.tile as tile
    from concourse import bacc

    F32 = mybir.dt.float32
    BF16 = mybir.dt.bfloat16
    FP8 = mybir.dt.float8e4
    AF = mybir.ActivationFunctionType
    DR = mybir.MatmulPerfMode.DoubleRow

    nc = bacc.Bacc(None, target_bir_lowering=False)
    xt_d = nc.declare_dram_parameter("xt", [D, T], FP8, isOutput=False)
    xtv_d = nc.declare_dram_parameter("xtv", [D, T], BF16, isOutput=False)
    wq_d = nc.declare_dram_parameter("wq", [D, 512], FP8, isOutput=False)
    wk_d = nc.declare_dram_parameter("wk", [D, 512], FP8, isOutput=False)
    wv_d = nc.declare_dram_parameter("wv", [D, 512], BF16, isOutput=False)
    wo_d = nc.declare_dram_parameter("wo", [512, D], BF16, isOutput=False)
    bias_d = nc.declare_dram_parameter("bias", [128, 8], F32, isOutput=False)
    mask_d = nc.declare_dram_parameter("mask", [128, 256], BF16, isOutput=False)
    yt_d = nc.declare_dram_parameter("yt", [D, T], BF16, isOutput=True)

    with tile.TileContext(nc) as tc:
        with (
            tc.tile_pool(name="persist", bufs=1) as pers,
            tc.tile_pool(name="work", bufs=1) as work,
            tc.tile_pool(name="dram", bufs=1, space="DRAM") as dram,
            tc.tile_pool(name="psum", bufs=1, space="PSUM") as psum,
        ):
            rsum_d = dram.tile([NP, 1024], F32)
            rec_d = dram.tile([NP, 1024], mybir.dt.float16)
            F16 = mybir.dt.float16
            rsT8 = pers.tile([128, NP, 8], F32)
            rec8 = pers.tile([128, NP, 8], F32)
            rec8b = pers.tile([128, NP, 8], F16)
            recrow = pers.tile([1, NP * 1024], F16)
            ones64 = pers.tile([1, 64], F16)
            qt = pers.tile([128, NP, T], BF16)
            kt = pers.tile([128, NP, T], BF16)
            v = pers.tile([128, TB, 8 * 65], BF16)
            ot = pers.tile([128, NP, T], BF16)
            mneg = pers.tile([128, 256], BF16)
            wo = pers.tile([128, 4, D], BF16)
            bias = pers.tile([128, 8], F32)
            wq = pers.tile([128, 8, 512], FP8)
            wk = pers.tile([128, 8, 512], FP8)
            wv = pers.tile([128, 8, 512], BF16)
            xsb = pers.tile([128, 8, T], FP8)
            xv = pers.tile([128, 8, T], BF16)

            # DMA priority order: operands of the first matmuls first.
            for c in range(8):
                nc.sync.dma_start(
                    out=xsb[:, c, 0:QW], in_=xt_d[c * 128:(c + 1) * 128, 0:QW]
                )
                nc.sync.dma_start(out=wq[:, c, :], in_=wq_d[c * 128:(c + 1) * 128, :])
            nc.sync.dma_start(out=mneg[:], in_=mask_d[:])
            nc.sync.dma_start(out=bias[:], in_=bias_d[:])
            for c in range(8):
                nc.sync.dma_start(out=wk[:, c, :], in_=wk_d[c * 128:(c + 1) * 128, :])
            for c in range(8):
                nc.sync.dma_start(
                    out=xv[:, c, 0:QW], in_=xtv_d[c * 128:(c + 1) * 128, 0:QW]
                )
                nc.sync.dma_start(out=wv[:, c, :], in_=wv_d[c * 128:(c + 1) * 128, :])
            for c in range(8):
                nc.sync.dma_start(
                    out=xsb[:, c, QW:T], in_=xt_d[c * 128:(c + 1) * 128, QW:T]
                )
            for c in range(8):
                nc.sync.dma_start(
                    out=xv[:, c, QW:T], in_=xtv_d[c * 128:(c + 1) * 128, QW:T]
                )
            for c in range(4):
                nc.sync.dma_start(out=wo[:, c, :], in_=wo_d[c * 128:(c + 1) * 128, :])
            nc.vector.memset(ones64[:], 1.0)
            # the V ones-columns never change: set once (SV cancels in normalize)
            nc.vector.memset(
                v[:].rearrange("p t (h c) -> p t h c", h=8)[:, :, :, 64:65], SV
            )
            # pre-warm the Exp activation table
            nc.scalar.activation(rsT8[0:1, 0, 0:1], bias[0:1, 0:1], AF.Exp)

            # Filler units: projection / output-projection work chopped into
            # single-instruction generator steps, drained one step at a time
            # into the tensor-idle bubbles of the attention block loop.
            def q_gen(j, p, w_sb=wq, dst=qt):
                jr = slice(j * QW, (j + 1) * QW)
                acc = psum.tile([128, QW], F32, tag="small", bufs=2)
                for cc in range(4):
                    nc.tensor.matmul(
                        acc[:],
                        w_sb[:, 2 * cc:2 * cc + 2, p * 128:(p + 1) * 128],
                        xsb[:, 2 * cc:2 * cc + 2, jr],
                        start=(cc == 0),
                        stop=(cc == 3),
                        perf_mode=DR,
                    )
                    yield
                nc.scalar.copy(dst[:, p, jr], acc[:])
                yield

            def k_gen(j, p):
                yield from q_gen(j, p, w_sb=wk, dst=kt)

            def v_gen(j, sub):
                i = 4 * j + sub
                ir = slice(i * 128, (i + 1) * 128)
                acc = psum.tile([128, QW], F32, tag="small", bufs=2)
                for c in range(8):
                    nc.tensor.matmul(
                        acc[:],
                        xv[:, c, ir],
                        wv[:, c, :],
                        start=(c == 0),
                        stop=(c == 7),
                    )
                    yield
                vblk = v[:, i, :].rearrange("p (h c) -> p h c", c=65)
                nc.scalar.copy(
                    vblk[:, :, 0:64],
                    acc[:].rearrange("p (h c) -> p h c", c=64),
                )
                yield

            def outproj_gen(j):
                jr = slice(j * QW, (j + 1) * QW)
                for n in range(8):
                    yps = psum.tile([128, QW], F32, tag="small", bufs=2)
                    for c in range(4):
                        nc.tensor.matmul(
                            yps[:],
                            wo[:, c, n * 128:(n + 1) * 128],
                            ot[:, c, jr],
                            start=(c == 0), stop=(c == 3),
                        )
                        yield
                    ysb = work.tile([128, QW], BF16, tag="ysb", bufs=3)
                    nc.vector.tensor_scalar_add(ysb[:], yps[:], bias[:, n:n + 1])
                    nc.sync.dma_start(
                        out=yt_d[n * 128:(n + 1) * 128, jr], in_=ysb[:]
                    )
                    yield

            pending = []

            def drain(n):
                while n > 0 and pending:
                    try:
                        next(pending[0])
                        n -= 1
                    except StopIteration:
                        pending.pop(0)

            def drain_all():
                while pending:
                    try:
                        next(pending[0])
                    except StopIteration:
                        pending.pop(0)

            def proj0():
                pending.extend([q_gen(0, p) for p in range(NP)])
                pending.extend([k_gen(0, p) for p in range(NP)])
                pending.extend([v_gen(0, sub) for sub in range(4)])
                drain_all()

            # --- normalize chain pieces (lag-1 pipelined) ---------------- #
            def norm_start(j, p, ocp):
                """rowsum row -> [128,8]: bounce through DRAM (linear DRAM
                APs absorb the transpose; SBUF<->SBUF cannot balance it).
                Dispatched from the otherwise-idle gpsimd queue: the SP
                queue's in-order dispatch would head-of-line-block these tiny
                hops behind yt stores waiting on their casts."""
                nc.gpsimd.dma_start(out=rsum_d[p:p + 1, :], in_=ocp[64:65, :])
                nc.gpsimd.dma_start(
                    out=rsT8[:, p, :],
                    in_=rsum_d[p:p + 1, :].rearrange("p (c q) -> p q c", c=8),
                )

            def norm_recip(j, p):
                """reciprocal in transposed form + return transposing hops."""
                nc.vector.reciprocal(rec8[:, p, :], rsT8[:, p, :])
                nc.vector.tensor_copy(rec8b[:, p, :], rec8[:, p, :])
                nc.gpsimd.dma_start(out=rec_d[p:p + 1, :], in_=rec8b[:, p, :])
                nc.gpsimd.dma_start(
                    out=recrow[0:1, p * 1024:(p + 1) * 1024],
                    in_=rec_d[p:p + 1, :].rearrange("p (q c) -> p c q", c=8),
                )

            def norm_finish(j, p, ocp):
                """PE outer-product broadcast + the 2 normalize muls."""
                jr = slice(j * QW, (j + 1) * QW)
                bc = psum.tile([128, 1024], F32, tag="st", bufs=2)
                for s in range(2):
                    nc.tensor.matmul(
                        bc[s * 64:(s + 1) * 64, s * QW:(s + 1) * QW],
                        ones64[:],
                        recrow[0:1, p * 1024 + s * QW:p * 1024 + (s + 1) * QW],
                        start=True, stop=True,
                    )
                for s in range(2):
                    nc.vector.tensor_mul(
                        ot[s * 64:(s + 1) * 64, p, jr],
                        ocp[0:64, s * QW:(s + 1) * QW],
                        bc[s * 64:(s + 1) * 64, s * QW:(s + 1) * QW],
                    )

            # normalize pipeline state: [(j, p, ocp), ...] of unfinished pairs.
            # recip at lag-1 (next pair's first block), broadcast+muls at
            # lag-2 (two pairs later) so the DMA bounce latency never gates
            # any engine.
            norm_q = []

            def attn_p(j, p, drain_n):
                jr = slice(j * QW, (j + 1) * QW)
                hA, hB = 2 * p, 2 * p + 1
                o_A = psum.tile([65, QW], F32, tag="o", bufs=2)
                o_B = psum.tile([65, QW], F32, tag="o", bufs=2)
                nkb = 4 * j + 4
                for kb in range(nkb):
                    o = kb - 4 * j  # diagonal offset; < 0 means full block
                    lo = 128 * o if o > 0 else 0  # first live q col in range
                    st = psum.tile([128, 1024], F32, tag="st", bufs=2)
                    kcols = slice(kb * 128, (kb + 1) * 128)
                    qcols = slice(j * QW + lo, (j + 1) * QW)
                    nc.tensor.matmul(
                        st[:, lo:QW],
                        kt[0:64, p, kcols],
                        qt[0:64, p, qcols],
                        start=True, stop=True, tile_position=(0, 0),
                    )
                    nc.tensor.matmul(
                        st[:, QW + lo:2 * QW],
                        kt[64:128, p, kcols],
                        qt[64:128, p, qcols],
                        start=True, stop=True, tile_position=(64, 0),
                    )
                    stv = st[:].rearrange("p (h q) -> p h q", h=2)
                    if o >= 0:
                        # additive causal mask on the diagonal 128-slab
                        nc.vector.tensor_add(
                            stv[:, :, lo:lo + 128],
                            stv[:, :, lo:lo + 128],
                            mneg[:].rearrange("p (h q) -> p h q", h=2),
                        )
                    pt = work.tile([128, 1024], BF16, tag="pt", bufs=3)
                    nc.scalar.activation(
                        pt[:].rearrange("p (h q) -> p h q", h=2)[:, :, lo:QW],
                        stv[:, :, lo:QW],
                        AF.Exp,
                        scale=EXP_SCALE,
                    )
                    drain(drain_n)  # fill the exp-wait bubble with fillers
                    nc.tensor.matmul(
                        o_A[:, lo:QW],
                        v[:, kb, hA * 65:(hA + 1) * 65],
                        pt[:, lo:QW],
                        start=(kb == 0), stop=(kb == nkb - 1),
                    )
                    nc.tensor.matmul(
                        o_B[:, lo:QW],
                        v[:, kb, hB * 65:(hB + 1) * 65],
                        pt[:, QW + lo:2 * QW],
                        start=(kb == 0), stop=(kb == nkb - 1),
                    )
                    if kb == 0 and norm_q:
                        # newest queued pair's reciprocal: DVE is idle early
                        # in a pair (mask adds are diagonal-only), so the
                        # wait on its transpose-DMA hop is harmless here.
                        pj, pp, pocp = norm_q[-1]
                        norm_recip(pj, pp)
                # copy psum accumulators out so the o slots free early;
                # DVE (not ACT: ACT paces the exp stream).
                ocp = work.tile([65, 1024], F32, tag="ocp", bufs=4)
                nc.vector.tensor_copy(ocp[:, 0:QW], o_A[:])
                nc.vector.tensor_copy(ocp[:, QW:1024], o_B[:])
                norm_start(j, p, ocp)
                norm_q.append((j, p, ocp))
                if len(norm_q) > 2:
                    norm_finish(*norm_q.pop(0))

            proj0()
            # Filler supply per attention window: proj(j+1) is queued during
            # attn(j) and must fully drain before attn(j+1) (in-order tensor
            # queue).  outproj(j) is queued two pairs into attn(j+1) so its
            # ot reads are emitted after the lag-2 normalize muls of (j, 3).
            DRAIN_N = {0: 3, 1: 3, 2: 3, 3: 1}
            for j in range(NJ):
                for p in range(NP):
                    if j + 1 < NJ:
                        pending.append(q_gen(j + 1, p))
                        pending.append(k_gen(j + 1, p))
                        if p == NP - 1:
                            pending.extend([v_gen(j + 1, sub) for sub in range(4)])
                    if j > 0 and p == 2:
                        pending.append(outproj_gen(j - 1))
                    attn_p(j, p, DRAIN_N[j])
                drain_all()
            # tail: finish the pipelined pairs, then the last outproj
            norm_recip(norm_q[-1][0], norm_q[-1][1])
            while norm_q:
                norm_finish(*norm_q.pop(0))
            pending.append(outproj_gen(3))
            drain_all()

    nc.finalize()
    return nc


def _prep_inputs(x, Wq, Wk, Wv, Wo, bo):
    """Build the 8 per-core input maps (host-side layout prep only)."""
    import ml_dtypes

    FP8 = ml_dtypes.float8_e4m3
    scale = np.float32(1.0 / np.sqrt(np.float32(HD)) * 2.0 ** SQ_EXP)
    kr = np.arange(128, dtype=np.float32)[:, None]
    qc = np.arange(128, dtype=np.float32)[None, :]
    tri = np.where(qc >= kr, np.float32(0.0), np.float32(MASK_VAL))
    mneg = np.tile(tri, (1, 2)).astype(ml_dtypes.bfloat16)

    in_maps = []
    for c in range(NCORES):
        b, g = c // 2, c % 2
        hs = slice(g * 8, (g + 1) * 8)
        xtf = np.ascontiguousarray(x[b].T)
        xt = xtf.astype(FP8)
        xtv = xtf.astype(ml_dtypes.bfloat16)
        wq = np.ascontiguousarray(Wq[hs].reshape(512, D).T * scale).astype(FP8)
        wk = np.ascontiguousarray(Wk[hs].reshape(512, D).T * np.float32(2.0 ** SK_EXP)).astype(FP8)
        wv = np.ascontiguousarray(Wv[hs].reshape(512, D).T).astype(ml_dtypes.bfloat16)
        wo = np.ascontiguousarray(Wo[:, g * 512:(g + 1) * 512].T).astype(ml_dtypes.bfloat16)
        if g == 0:
            bias = np.ascontiguousarray(bo.reshape(8, 128).T)
        else:
            bias = np.zeros((128, 8), np.float32)
        in_maps.append(
            {"xt": xt, "xtv": xtv, "wq": wq, "wk": wk, "wv": wv, "wo": wo,
             "bias": bias, "mask": mneg}
        )
    return in_maps


def _run(inputs, trace=False, trace_cores=None):
    from concourse.bass_utils import run_bass_kernel_spmd

    if "nc" not in _CACHE:
        _CACHE["nc"] = _build_nc()
    nc = _CACHE["nc"]
    in_maps = _prep_inputs(
        inputs["x"], inputs["Wq"], inputs["Wk"], inputs["Wv"], inputs["Wo"], inputs["bo"]
    )
    r = run_bass_kernel_spmd(
        nc, in_maps, list(range(NCORES)), trace=trace, trace_cores=trace_cores
    )
    y = np.empty((B, T, D), np.float32)
    for b in range(B):
        yt = np.asarray(r.results[2 * b]["yt"], dtype=np.float32) + np.asarray(
            r.results[2 * b + 1]["yt"], dtype=np.float32
        )
        y[b] = yt.T
    return y, r


def kernel(**inputs):
    y, _ = _run(inputs, trace=False)
    return y


# revision 25
# speedup vs baseline: 1.2963x; 1.1886x over previous
"""Trainium2 Bass kernel for causal multi-head attention (B=4, T=2048, D=1024, H=16).

Sharding: 8 cores = 4 batches x 2 head-groups (8 heads each), no collectives:
each core emits a full-[1024, T] bf16 partial of the output projection and the
host sums the pair per batch (the TP "unshard").

Per core (Tile framework, single SPMD program):
  proj(j):  Q/K/V projections in fp8e4 DoubleRow (2 fp8 MACs/cell/cycle,
            contraction 1024 = 4 x [128,2] chunks).  Weights carry power-of-2
            scales (wq 2^12 incl 1/sqrt(hd), wk 2^6, wv 2^6) so fp8 values sit
            in e4m3's sweet spot; the exp activation's scale compensates
            (2^-18) and the V ones-column is 64.0 (cancels in normalize).
  attn(j,p): causal flash attention in transposed layout, bf16 (HD=64 gives
            no fp8 win: DoubleRow needs contraction >= 256).  ST[k,q] =
            KT^T @ QT (head pair packed via tile_position), additive mask on
            the diagonal slab (DVE, pre-exp), PT = exp(ST * 2^-18) (ACT),
            O[hd+1, q] += [V|64]^T @ PT.
  normalize: rowsum row [1,1024] (from the V ones-column) -> one SBUF->SBUF
            transposing DMA -> [128,8] -> DVE reciprocal (parallel lanes) ->
            bf16 cast -> transposing DMA back to row form -> PE outer-product
            broadcast (ones[1,64] x recrow) into PSUM -> 2 DVE muls.
            The chain is lag-1 pipelined: pair p's reciprocal+return-hop are
            emitted after pair p+1's first block (DVE is idle there: the only
            DVE work early in a pair is nothing - mask adds are diagonal-only
            and proj casts moved to ACT), and the broadcast+muls at pair
            p+1's end (by which time the DMA hops have landed).
  outproj(j): YT[dout, q] = Wo_g^T @ OT + bias (g=0 adds bo), bf16.

Scheduling: proj/outproj work is chopped into single-instruction generator
fillers drained into the attention blocks' tensor bubbles (exp at 1 elem/
lane/cycle outpaces the 2+2 matmuls per block by ~0.25us).  proj(j+1) drains
during attn(j); outproj(j) drains one j later (its ot inputs are written by
the lag-1 muls).  PSUM->SBUF proj casts run on the ACT engine so the tensor
engine's resume at j-boundaries never sits behind normalize work on DVE.

hw quirks inherited from the baseline: custom-DVE ops silently corrupt on
this runtime; gpsimd is slow (~1us/op) except memset; engine partition
offsets must be multiples of 32; DVE ops allow per-operand partition BASE
offsets (sizes must match); matmul PSUM output must stay within one 2KB
bank (N <= 512 fp32).

Host: transpose/slice/scale weights to fp8/bf16, sum per-core partials,
assemble [B, T, D].
"""

import numpy as np

B, T, D, H, HD = 4, 2048, 1024, 16, 64
NCORES = 8
NP = 4          # head pairs per core
NJ = 4          # q-ranges of 512
QW = 512
TB = T // 128   # 16

# power-of-2 scale folding for the fp8 Q/K projections
SQ_EXP = 12     # wq scale 2^12 (on top of 1/sqrt(HD))
SK_EXP = 6      # wk scale 2^6
SV = 1.0        # V path is bf16: no scale; ones column stays 1.0
EXP_SCALE = 2.0 ** (-(SQ_EXP + SK_EXP))
MASK_VAL = -50.0 * 2.0 ** (SQ_EXP + SK_EXP)

_CACHE = {}


def _build_nc():
    import concourse.mybir as mybir
    import concourse.tile as tile
    from concourse import bacc

    F32 = mybir.dt.float32
    BF16 = mybir.dt.bfloat16
    FP8 = mybir.dt.float8e4
    AF = mybir.ActivationFunctionType
    DR = mybir.MatmulPerfMode.DoubleRow

    nc = bacc.Bacc(None, target_bir_lowering=False)
    xt_d = nc.declare_dram_parameter("xt", [D, T], FP8, isOutput=False)
    xtv_d = nc.declare_dram_parameter("xtv", [D, T], BF16, isOutput=False)
    wq_d = nc.declare_dram_parameter("wq", [D, 512], FP8, isOutput=False)
    wk_d = nc.declare_dram_parameter("wk", [D, 512], FP8, isOutput=False)
    wv_d = nc.declare_dram_parameter("wv", [D, 512], BF16, isOutput=False)
    wo_d = nc.declare_dram_parameter("wo", [512, D], BF16, isOutput=False)
    bias_d = nc.declare_dram_parameter("bias", [128, 8], F32, isOutput=False)
    mask_d = nc.declare_dram_parameter("mask", [128, 256], BF16, isOutput=False)
    yt_d = nc.declare_dram_parameter("yt", [D, T], BF16, isOutput=True)

    with tile.TileContext(nc) as tc:
        with (
            tc.tile_pool(name="persist", bufs=1) as pers,
            tc.tile_pool(name="work", bufs=1) as work,
            tc.tile_pool(name="psum", bufs=1, space="PSUM") as psum,
        ):
            F16 = mybir.dt.float16
            lnrow = pers.tile([1, NP * 1024], F32)
            recrow = pers.tile([1, NP * 1024], F16)
            ones64 = pers.tile([1, 64], F16)
            warm = pers.tile([1, 1], F32)
            qt = pers.tile([128, NP, T], BF16)
            kt = pers.tile([128, NP, T], BF16)
            v = pers.tile([128, TB, 8 * 65], BF16)
            ot = pers.tile([128, NP, T], BF16)
            mneg = pers.tile([128, 256], BF16)
            wo = pers.tile([128, 4, D], BF16)
            bias = pers.tile([128, 8], F32)
            wq = pers.tile([128, 8, 512], FP8)
            wk = pers.tile([128, 8, 512], FP8)
            wv = pers.tile([128, 8, 512], BF16)
            xsb = pers.tile([128, 8, T], FP8)
            xv = pers.tile([128, 8, T], BF16)

            # DMA priority order: operands of the first matmuls first.
            for c in range(8):
                nc.sync.dma_start(
                    out=xsb[:, c, 0:QW], in_=xt_d[c * 128:(c + 1) * 128, 0:QW]
                )
                nc.sync.dma_start(out=wq[:, c, :], in_=wq_d[c * 128:(c + 1) * 128, :])
            nc.sync.dma_start(out=mneg[:], in_=mask_d[:])
            nc.sync.dma_start(out=bias[:], in_=bias_d[:])
            for c in range(8):
                nc.sync.dma_start(out=wk[:, c, :], in_=wk_d[c * 128:(c + 1) * 128, :])
            for c in range(8):
                nc.sync.dma_start(
                    out=xv[:, c, 0:QW], in_=xtv_d[c * 128:(c + 1) * 128, 0:QW]
                )
                nc.sync.dma_start(out=wv[:, c, :], in_=wv_d[c * 128:(c + 1) * 128, :])
            for c in range(8):
                nc.sync.dma_start(
                    out=xsb[:, c, QW:T], in_=xt_d[c * 128:(c + 1) * 128, QW:T]
                )
            for c in range(8):
                nc.sync.dma_start(
                    out=xv[:, c, QW:T], in_=xtv_d[c * 128:(c + 1) * 128, QW:T]
                )
            for c in range(4):
                nc.sync.dma_start(out=wo[:, c, :], in_=wo_d[c * 128:(c + 1) * 128, :])
            nc.vector.memset(ones64[:], 1.0)
            # the V ones-columns never change: set once (SV cancels in normalize)
            nc.vector.memset(
                v[:].rearrange("p t (h c) -> p t h c", h=8)[:, :, :, 64:65], SV
            )
            # pre-warm the Exp and Ln activation tables
            nc.scalar.activation(warm[0:1, 0:1], bias[0:1, 0:1], AF.Exp)
            nc.scalar.activation(warm[0:1, 0:1], bias[0:1, 0:1], AF.Ln)

            # Filler units: projection / output-projection work chopped into
            # single-instruction generator steps, drained one step at a time
            # into the tensor-idle bubbles of the attention block loop.
            def q_gen(j, p, w_sb=wq, dst=qt):
                jr = slice(j * QW, (j + 1) * QW)
                acc = psum.tile([128, QW], F32, tag="small", bufs=2)
                for cc in range(4):
                    nc.tensor.matmul(
                        acc[:],
                        w_sb[:, 2 * cc:2 * cc + 2, p * 128:(p + 1) * 128],
                        xsb[:, 2 * cc:2 * cc + 2, jr],
                        start=(cc == 0),
                        stop=(cc == 3),
                        perf_mode=DR,
                    )
                    yield
                nc.vector.tensor_copy(dst[:, p, jr], acc[:])
                yield

            def k_gen(j, p):
                yield from q_gen(j, p, w_sb=wk, dst=kt)

            def v_gen(j, sub):
                i = 4 * j + sub
                ir = slice(i * 128, (i + 1) * 128)
                acc = psum.tile([128, QW], F32, tag="small", bufs=2)
                for c in range(8):
                    nc.tensor.matmul(
                        acc[:],
                        xv[:, c, ir],
                        wv[:, c, :],
                        start=(c == 0),
                        stop=(c == 7),
                    )
                    yield
                vblk = v[:, i, :].rearrange("p (h c) -> p h c", c=65)
                nc.vector.tensor_copy(
                    vblk[:, :, 0:64],
                    acc[:].rearrange("p (h c) -> p h c", c=64),
                )
                yield

            def outproj_gen(j):
                jr = slice(j * QW, (j + 1) * QW)
                for n in range(8):
                    yps = psum.tile([128, QW], F32, tag="small", bufs=2)
                    for c in range(4):
                        nc.tensor.matmul(
                            yps[:],
                            wo[:, c, n * 128:(n + 1) * 128],
                            ot[:, c, jr],
                            start=(c == 0), stop=(c == 3),
                        )
                        yield
                    ysb = work.tile([128, QW], BF16, tag="ysb", bufs=3)
                    nc.vector.tensor_scalar_add(ysb[:], yps[:], bias[:, n:n + 1])
                    nc.sync.dma_start(
                        out=yt_d[n * 128:(n + 1) * 128, jr], in_=ysb[:]
                    )
                    yield

            pending = []

            def drain(n):
                while n > 0 and pending:
                    try:
                        next(pending[0])
                        n -= 1
                    except StopIteration:
                        pending.pop(0)

            def drain_all():
                while pending:
                    try:
                        next(pending[0])
                    except StopIteration:
                        pending.pop(0)

            def proj0():
                pending.extend([q_gen(0, p) for p in range(NP)])
                pending.extend([k_gen(0, p) for p in range(NP)])
                pending.extend([v_gen(0, sub) for sub in range(4)])
                drain_all()

            # --- normalize chain pieces (lag-1 pipelined) ---------------- #
            def norm_recip(j, p, ocp):
                """reciprocal of the rowsum row as exp(-ln(x)) on the ACT
                engine: a [1,1024] row is one serial lane there (~1.1us/op),
                and the ln/exp tables are co-resident -- no DMA, no
                transposes, no cross-queue serialization."""
                ps = slice(p * 1024, (p + 1) * 1024)
                nc.scalar.activation(lnrow[0:1, ps], ocp[64:65, :], AF.Ln)
                nc.scalar.activation(recrow[0:1, ps], lnrow[0:1, ps], AF.Exp,
                                     scale=-1.0)

            def norm_finish(j, p, ocp):
                """PE outer-product broadcast + the 2 normalize muls."""
                jr = slice(j * QW, (j + 1) * QW)
                bc = psum.tile([128, 1024], F32, tag="st", bufs=2)
                for s in range(2):
                    nc.tensor.matmul(
                        bc[s * 64:(s + 1) * 64, s * QW:(s + 1) * QW],
                        ones64[:],
                        recrow[0:1, p * 1024 + s * QW:p * 1024 + (s + 1) * QW],
                        start=True, stop=True,
                    )
                for s in range(2):
                    nc.vector.tensor_mul(
                        ot[s * 64:(s + 1) * 64, p, jr],
                        ocp[0:64, s * QW:(s + 1) * QW],
                        bc[s * 64:(s + 1) * 64, s * QW:(s + 1) * QW],
                    )

            # normalize pipeline state: [(j, p, ocp), ...] of unfinished
            # pairs.  reciprocal at lag-1 (next pair's first block),
            # broadcast+muls at the next pair's end.
            norm_q = []

            def attn_p(j, p, drain_n):
                jr = slice(j * QW, (j + 1) * QW)
                hA, hB = 2 * p, 2 * p + 1
                o_A = psum.tile([65, QW], F32, tag="o", bufs=2)
                o_B = psum.tile([65, QW], F32, tag="o", bufs=2)
                nkb = 4 * j + 4
                for kb in range(nkb):
                    o = kb - 4 * j  # diagonal offset; < 0 means full block
                    lo = 128 * o if o > 0 else 0  # first live q col in range
                    st = psum.tile([128, 1024], F32, tag="st", bufs=2)
                    kcols = slice(kb * 128, (kb + 1) * 128)
                    qcols = slice(j * QW + lo, (j + 1) * QW)
                    nc.tensor.matmul(
                        st[:, lo:QW],
                        kt[0:64, p, kcols],
                        qt[0:64, p, qcols],
                        start=True, stop=True, tile_position=(0, 0),
                    )
                    nc.tensor.matmul(
                        st[:, QW + lo:2 * QW],
                        kt[64:128, p, kcols],
                        qt[64:128, p, qcols],
                        start=True, stop=True, tile_position=(64, 0),
                    )
                    stv = st[:].rearrange("p (h q) -> p h q", h=2)
                    if o >= 0:
                        # additive causal mask on the diagonal 128-slab
                        nc.vector.tensor_add(
                            stv[:, :, lo:lo + 128],
                            stv[:, :, lo:lo + 128],
                            mneg[:].rearrange("p (h q) -> p h q", h=2),
                        )
                    pt = work.tile([128, 1024], BF16, tag="pt", bufs=3)
                    nc.scalar.activation(
                        pt[:].rearrange("p (h q) -> p h q", h=2)[:, :, lo:QW],
                        stv[:, :, lo:QW],
                        AF.Exp,
                        scale=EXP_SCALE,
                    )
                    drain(drain_n)  # fill the exp-wait bubble with fillers
                    nc.tensor.matmul(
                        o_A[:, lo:QW],
                        v[:, kb, hA * 65:(hA + 1) * 65],
                        pt[:, lo:QW],
                        start=(kb == 0), stop=(kb == nkb - 1),
                    )
                    nc.tensor.matmul(
                        o_B[:, lo:QW],
                        v[:, kb, hB * 65:(hB + 1) * 65],
                        pt[:, QW + lo:2 * QW],
                        start=(kb == 0), stop=(kb == nkb - 1),
                    )
                    if kb == 0 and norm_q:
                        # previous pair's reciprocal, placed one block into
                        # this pair so its ocp-copy wait is already met.
                        norm_recip(*norm_q[-1])
                # copy psum accumulators out so the o slots free early
                ocp = work.tile([65, 1024], F32, tag="ocp", bufs=4)
                nc.vector.tensor_copy(ocp[:, 0:QW], o_A[:])
                nc.vector.tensor_copy(ocp[:, QW:1024], o_B[:])
                norm_q.append((j, p, ocp))
                if len(norm_q) > 1:
                    norm_finish(*norm_q.pop(0))

            proj0()
            # Filler supply per attention window: proj(j+1) is queued during
            # attn(j) and must fully drain before attn(j+1) (in-order tensor
            # queue).  outproj(j) is queued two pairs into attn(j+1) so its
            # ot reads are emitted after the lag-2 normalize muls of (j, 3).
            DRAIN_N = {0: 3, 1: 3, 2: 3, 3: 1}
            for j in range(NJ):
                for p in range(NP):
                    if j + 1 < NJ:
                        pending.append(q_gen(j + 1, p))
                        pending.append(k_gen(j + 1, p))
                        if p == NP - 1:
                            pending.extend([v_gen(j + 1, sub) for sub in range(4)])
                    if j > 0 and p == 1:
                        pending.append(outproj_gen(j - 1))
                    attn_p(j, p, DRAIN_N[j])
                drain_all()
            # tail: finish the last pair, then the last outproj
            norm_recip(*norm_q[-1])
            norm_finish(*norm_q.pop(0))
            pending.append(outproj_gen(3))
            drain_all()

    nc.finalize()
    return nc


def _prep_inputs(x, Wq, Wk, Wv, Wo, bo):
    """Build the 8 per-core input maps (host-side layout prep only)."""
    import ml_dtypes

    FP8 = ml_dtypes.float8_e4m3
    scale = np.float32(1.0 / np.sqrt(np.float32(HD)) * 2.0 ** SQ_EXP)
    kr = np.arange(128, dtype=np.float32)[:, None]
    qc = np.arange(128, dtype=np.float32)[None, :]
    tri = np.where(qc >= kr, np.float32(0.0), np.float32(MASK_VAL))
    mneg = np.tile(tri, (1, 2)).astype(ml_dtypes.bfloat16)

    in_maps = []
    for c in range(NCORES):
        b, g = c // 2, c % 2
        hs = slice(g * 8, (g + 1) * 8)
        xtf = np.ascontiguousarray(x[b].T)
        xt = xtf.astype(FP8)
        xtv = xtf.astype(ml_dtypes.bfloat16)
        wq = np.ascontiguousarray(Wq[hs].reshape(512, D).T * scale).astype(FP8)
        wk = np.ascontiguousarray(Wk[hs].reshape(512, D).T * np.float32(2.0 ** SK_EXP)).astype(FP8)
        wv = np.ascontiguousarray(Wv[hs].reshape(512, D).T).astype(ml_dtypes.bfloat16)
        wo = np.ascontiguousarray(Wo[:, g * 512:(g + 1) * 512].T).astype(ml_dtypes.bfloat16)
        if g == 0:
            bias = np.ascontiguousarray(bo.reshape(8, 128).T)
        else:
            bias = np.zeros((128, 8), np.float32)
        in_maps.append(
            {"xt": xt, "xtv": xtv, "wq": wq, "wk": wk, "wv": wv, "wo": wo,
             "bias": bias, "mask": mneg}
        )
    return in_maps


def _run(inputs, trace=False, trace_cores=None):
    from concourse.bass_utils import run_bass_kernel_spmd

    if "nc" not in _CACHE:
        _CACHE["nc"] = _build_nc()
    nc = _CACHE["nc"]
    in_maps = _prep_inputs(
        inputs["x"], inputs["Wq"], inputs["Wk"], inputs["Wv"], inputs["Wo"], inputs["bo"]
    )
    r = run_bass_kernel_spmd(
        nc, in_maps, list(range(NCORES)), trace=trace, trace_cores=trace_cores
    )
    y = np.empty((B, T, D), np.float32)
    for b in range(B):
        yt = np.asarray(r.results[2 * b]["yt"], dtype=np.float32) + np.asarray(
            r.results[2 * b + 1]["yt"], dtype=np.float32
        )
        y[b] = yt.T
    return y, r


def kernel(**inputs):
    y, _ = _run(inputs, trace=False)
    return y


# revision 29
# speedup vs baseline: 1.3038x; 1.0057x over previous
"""Trainium2 Bass kernel for causal multi-head attention (B=4, T=2048, D=1024, H=16).

Sharding: 8 cores = 4 batches x 2 head-groups (8 heads each), no collectives:
each core emits a full-[1024, T] bf16 partial of the output projection and the
host sums the pair per batch (the TP "unshard").

Per core (Tile framework, single SPMD program):
  proj(j):  Q/K/V projections in fp8e4 DoubleRow (2 fp8 MACs/cell/cycle,
            contraction 1024 = 4 x [128,2] chunks).  Weights carry power-of-2
            scales (wq 2^12 incl 1/sqrt(hd), wk 2^6, wv 2^6) so fp8 values sit
            in e4m3's sweet spot; the exp activation's scale compensates
            (2^-18) and the V ones-column is 64.0 (cancels in normalize).
  attn(j,p): causal flash attention in transposed layout, bf16 (HD=64 gives
            no fp8 win: DoubleRow needs contraction >= 256).  ST[k,q] =
            KT^T @ QT (head pair packed via tile_position), additive mask on
            the diagonal slab (DVE, pre-exp), PT = exp(ST * 2^-18) (ACT),
            O[hd+1, q] += [V|64]^T @ PT.
  normalize: rowsum row [1,1024] (from the V ones-column) -> one SBUF->SBUF
            transposing DMA -> [128,8] -> DVE reciprocal (parallel lanes) ->
            bf16 cast -> transposing DMA back to row form -> PE outer-product
            broadcast (ones[1,64] x recrow) into PSUM -> 2 DVE muls.
            The chain is lag-1 pipelined: pair p's reciprocal+return-hop are
            emitted after pair p+1's first block (DVE is idle there: the only
            DVE work early in a pair is nothing - mask adds are diagonal-only
            and proj casts moved to ACT), and the broadcast+muls at pair
            p+1's end (by which time the DMA hops have landed).
  outproj(j): YT[dout, q] = Wo_g^T @ OT + bias (g=0 adds bo), bf16.

Scheduling: proj/outproj work is chopped into single-instruction generator
fillers drained into the attention blocks' tensor bubbles (exp at 1 elem/
lane/cycle outpaces the 2+2 matmuls per block by ~0.25us).  proj(j+1) drains
during attn(j); outproj(j) drains one j later (its ot inputs are written by
the lag-1 muls).  PSUM->SBUF proj casts run on the ACT engine so the tensor
engine's resume at j-boundaries never sits behind normalize work on DVE.

hw quirks inherited from the baseline: custom-DVE ops silently corrupt on
this runtime; gpsimd is slow (~1us/op) except memset; engine partition
offsets must be multiples of 32; DVE ops allow per-operand partition BASE
offsets (sizes must match); matmul PSUM output must stay within one 2KB
bank (N <= 512 fp32).

Host: transpose/slice/scale weights to fp8/bf16, sum per-core partials,
assemble [B, T, D].
"""

import numpy as np

B, T, D, H, HD = 4, 2048, 1024, 16, 64
NCORES = 8
NP = 4          # head pairs per core
NJ = 4          # q-ranges of 512
QW = 512
TB = T // 128   # 16

# power-of-2 scale folding for the fp8 Q/K projections
SQ_EXP = 12     # wq scale 2^12 (on top of 1/sqrt(HD))
SK_EXP = 6      # wk scale 2^6
SV = 1.0        # V path is bf16: no scale; ones column stays 1.0
EXP_SCALE = 2.0 ** (-(SQ_EXP + SK_EXP))
MASK_VAL = -50.0 * 2.0 ** (SQ_EXP + SK_EXP)

_CACHE = {}


def _build_nc():
    import concourse.mybir as mybir
    import concourse.tile as tile
    from concourse import bacc

    F32 = mybir.dt.float32
    BF16 = mybir.dt.bfloat16
    FP8 = mybir.dt.float8e4
    AF = mybir.ActivationFunctionType
    DR = mybir.MatmulPerfMode.DoubleRow

    nc = bacc.Bacc(None, target_bir_lowering=False)
    xt_d = nc.declare_dram_parameter("xt", [D, T], FP8, isOutput=False)
    xtv_d = nc.declare_dram_parameter("xtv", [D, T], BF16, isOutput=False)
    wq_d = nc.declare_dram_parameter("wq", [D, 512], FP8, isOutput=False)
    wk_d = nc.declare_dram_parameter("wk", [D, 512], FP8, isOutput=False)
    wv_d = nc.declare_dram_parameter("wv", [D, 512], BF16, isOutput=False)
    wo_d = nc.declare_dram_parameter("wo", [512, D], BF16, isOutput=False)
    bias_d = nc.declare_dram_parameter("bias", [128, 8], F32, isOutput=False)
    mask_d = nc.declare_dram_parameter("mask", [128, 256], BF16, isOutput=False)
    yt_d = nc.declare_dram_parameter("yt", [D, T], BF16, isOutput=True)

    with tile.TileContext(nc) as tc:
        with (
            tc.tile_pool(name="persist", bufs=1) as pers,
            tc.tile_pool(name="work", bufs=1) as work,
            tc.tile_pool(name="psum", bufs=1, space="PSUM") as psum,
        ):
            F16 = mybir.dt.float16
            lnrow = pers.tile([1, NP * 1024], F32)
            recrow = pers.tile([1, NP * 1024], F16)
            ones64 = pers.tile([1, 64], F16)
            warm = pers.tile([1, 1], F32)
            qt = pers.tile([128, NP, T], BF16)
            kt = pers.tile([128, NP, T], BF16)
            v = pers.tile([128, TB, 8 * 65], BF16)
            ot = pers.tile([128, NP, T], BF16)
            mneg = pers.tile([128, 256], BF16)
            wo = pers.tile([128, 4, D], BF16)
            bias = pers.tile([128, 8], F32)
            wq = pers.tile([128, 8, 512], FP8)
            wk = pers.tile([128, 8, 512], FP8)
            wv = pers.tile([128, 8, 512], BF16)
            xsb = pers.tile([128, 8, T], FP8)
            xv = pers.tile([128, 8, T], BF16)

            # DMA priority order: operands of the first matmuls first.
            for c in range(8):
                nc.sync.dma_start(
                    out=xsb[:, c, 0:QW], in_=xt_d[c * 128:(c + 1) * 128, 0:QW]
                )
                nc.sync.dma_start(out=wq[:, c, :], in_=wq_d[c * 128:(c + 1) * 128, :])
            nc.sync.dma_start(out=mneg[:], in_=mask_d[:])
            nc.sync.dma_start(out=bias[:], in_=bias_d[:])
            for c in range(8):
                nc.sync.dma_start(out=wk[:, c, :], in_=wk_d[c * 128:(c + 1) * 128, :])
            for c in range(8):
                nc.sync.dma_start(
                    out=xv[:, c, 0:QW], in_=xtv_d[c * 128:(c + 1) * 128, 0:QW]
                )
                nc.sync.dma_start(out=wv[:, c, :], in_=wv_d[c * 128:(c + 1) * 128, :])
            for c in range(8):
                nc.sync.dma_start(
                    out=xsb[:, c, QW:T], in_=xt_d[c * 128:(c + 1) * 128, QW:T]
                )
            for c in range(8):
                nc.sync.dma_start(
                    out=xv[:, c, QW:T], in_=xtv_d[c * 128:(c + 1) * 128, QW:T]
                )
            for c in range(4):
                nc.sync.dma_start(out=wo[:, c, :], in_=wo_d[c * 128:(c + 1) * 128, :])
            nc.vector.memset(ones64[:], 1.0)
            # the V ones-columns never change: set once (SV cancels in normalize)
            nc.vector.memset(
                v[:].rearrange("p t (h c) -> p t h c", h=8)[:, :, :, 64:65], SV
            )
            # pre-warm the Exp and Ln activation tables (exp output is
            # positive, so it is a safe ln input)
            nc.scalar.activation(warm[0:1, 0:1], bias[0:1, 0:1], AF.Exp)
            nc.scalar.activation(warm[0:1, 0:1], warm[0:1, 0:1], AF.Ln)

            # Filler units: projection / output-projection work chopped into
            # single-instruction generator steps, drained one step at a time
            # into the tensor-idle bubbles of the attention block loop.
            def q_gen(j, p, w_sb=wq, dst=qt):
                jr = slice(j * QW, (j + 1) * QW)
                acc = psum.tile([128, QW], F32, tag="small", bufs=2)
                for cc in range(4):
                    nc.tensor.matmul(
                        acc[:],
                        w_sb[:, 2 * cc:2 * cc + 2, p * 128:(p + 1) * 128],
                        xsb[:, 2 * cc:2 * cc + 2, jr],
                        start=(cc == 0),
                        stop=(cc == 3),
                        perf_mode=DR,
                    )
                    yield
                nc.vector.tensor_copy(dst[:, p, jr], acc[:])
                yield

            def k_gen(j, p):
                yield from q_gen(j, p, w_sb=wk, dst=kt)

            def v_gen(j, sub):
                i = 4 * j + sub
                ir = slice(i * 128, (i + 1) * 128)
                acc = psum.tile([128, QW], F32, tag="small", bufs=2)
                for c in range(8):
                    nc.tensor.matmul(
                        acc[:],
                        xv[:, c, ir],
                        wv[:, c, :],
                        start=(c == 0),
                        stop=(c == 7),
                    )
                    yield
                vblk = v[:, i, :].rearrange("p (h c) -> p h c", c=65)
                nc.vector.tensor_copy(
                    vblk[:, :, 0:64],
                    acc[:].rearrange("p (h c) -> p h c", c=64),
                )
                yield

            def outproj_gen(j):
                jr = slice(j * QW, (j + 1) * QW)
                for n in range(8):
                    yps = psum.tile([128, QW], F32, tag="small", bufs=2)
                    for c in range(4):
                        nc.tensor.matmul(
                            yps[:],
                            wo[:, c, n * 128:(n + 1) * 128],
                            ot[:, c, jr],
                            start=(c == 0), stop=(c == 3),
                        )
                        yield
                    ysb = work.tile([128, QW], BF16, tag="ysb", bufs=3)
                    nc.vector.tensor_scalar_add(ysb[:], yps[:], bias[:, n:n + 1])
                    nc.sync.dma_start(
                        out=yt_d[n * 128:(n + 1) * 128, jr], in_=ysb[:]
                    )
                    yield

            pending = []
            started = [False]  # has pending[0] emitted any step yet?

            def drain(n, spread=True):
                # gen i's PSUM slot is reused by gen i+2 (small ring bufs=2),
                # so a fresh gen starting right after another ends would WAR-
                # wait on a cast the DVE may not have reached yet.  Defer a
                # fresh gen's first step to the next block (spread=True).
                emitted = 0
                while n > 0 and pending:
                    if spread and not started[0] and emitted > 0:
                        return
                    try:
                        next(pending[0])
                        started[0] = True
                        n -= 1
                        emitted += 1
                    except StopIteration:
                        pending.pop(0)
                        started[0] = False

            def drain_all():
                while pending:
                    try:
                        next(pending[0])
                    except StopIteration:
                        pending.pop(0)
                started[0] = False

            def proj0():
                pending.extend([q_gen(0, p) for p in range(NP)])
                pending.extend([k_gen(0, p) for p in range(NP)])
                pending.extend([v_gen(0, sub) for sub in range(4)])
                drain_all()

            # --- normalize chain pieces (lag-1 pipelined) ---------------- #
            def norm_recip(j, p, ocp):
                """reciprocal of the rowsum row as exp(-ln(x)) on the ACT
                engine: a [1,1024] row is one serial lane there (~1.1us/op),
                and the ln/exp tables are co-resident -- no DMA, no
                transposes, no cross-queue serialization."""
                ps = slice(p * 1024, (p + 1) * 1024)
                nc.scalar.activation(lnrow[0:1, ps], ocp[64:65, :], AF.Ln)
                nc.scalar.activation(recrow[0:1, ps], lnrow[0:1, ps], AF.Exp,
                                     scale=-1.0)

            def norm_finish(j, p, ocp):
                """PE outer-product broadcast + the 2 normalize muls."""
                jr = slice(j * QW, (j + 1) * QW)
                bc = psum.tile([128, 1024], F32, tag="st", bufs=2)
                for s in range(2):
                    nc.tensor.matmul(
                        bc[s * 64:(s + 1) * 64, s * QW:(s + 1) * QW],
                        ones64[:],
                        recrow[0:1, p * 1024 + s * QW:p * 1024 + (s + 1) * QW],
                        start=True, stop=True,
                    )
                for s in range(2):
                    nc.vector.tensor_mul(
                        ot[s * 64:(s + 1) * 64, p, jr],
                        ocp[0:64, s * QW:(s + 1) * QW],
                        bc[s * 64:(s + 1) * 64, s * QW:(s + 1) * QW],
                    )

            # normalize pipeline state: [(j, p, ocp), ...] of unfinished
            # pairs.  reciprocal at lag-1 (next pair's first block),
            # broadcast+muls at the next pair's end.
            norm_q = []

            def attn_p(j, p, drain_n):
                jr = slice(j * QW, (j + 1) * QW)
                hA, hB = 2 * p, 2 * p + 1
                o_A = psum.tile([65, QW], F32, tag="o", bufs=2)
                o_B = psum.tile([65, QW], F32, tag="o", bufs=2)
                nkb = 4 * j + 4
                for kb in range(nkb):
                    o = kb - 4 * j  # diagonal offset; < 0 means full block
                    lo = 128 * o if o > 0 else 0  # first live q col in range
                    st = psum.tile([128, 1024], F32, tag="st", bufs=2)
                    kcols = slice(kb * 128, (kb + 1) * 128)
                    qcols = slice(j * QW + lo, (j + 1) * QW)
                    nc.tensor.matmul(
                        st[:, lo:QW],
                        kt[0:64, p, kcols],
                        qt[0:64, p, qcols],
                        start=True, stop=True, tile_position=(0, 0),
                    )
                    nc.tensor.matmul(
                        st[:, QW + lo:2 * QW],
                        kt[64:128, p, kcols],
                        qt[64:128, p, qcols],
                        start=True, stop=True, tile_position=(64, 0),
                    )
                    stv = st[:].rearrange("p (h q) -> p h q", h=2)
                    if o >= 0:
                        # additive causal mask on the diagonal 128-slab
                        nc.vector.tensor_add(
                            stv[:, :, lo:lo + 128],
                            stv[:, :, lo:lo + 128],
                            mneg[:].rearrange("p (h q) -> p h q", h=2),
                        )
                    pt = work.tile([128, 1024], BF16, tag="pt", bufs=3)
                    nc.scalar.activation(
                        pt[:].rearrange("p (h q) -> p h q", h=2)[:, :, lo:QW],
                        stv[:, :, lo:QW],
                        AF.Exp,
                        scale=EXP_SCALE,
                    )
                    drain(drain_n)  # fill the exp-wait bubble with fillers
                    nc.tensor.matmul(
                        o_A[:, lo:QW],
                        v[:, kb, hA * 65:(hA + 1) * 65],
                        pt[:, lo:QW],
                        start=(kb == 0), stop=(kb == nkb - 1),
                    )
                    nc.tensor.matmul(
                        o_B[:, lo:QW],
                        v[:, kb, hB * 65:(hB + 1) * 65],
                        pt[:, QW + lo:2 * QW],
                        start=(kb == 0), stop=(kb == nkb - 1),
                    )
                    if kb == 0 and norm_q:
                        # previous pair's reciprocal, placed one block into
                        # this pair so its ocp-copy wait is already met.
                        norm_recip(*norm_q[-1])
                # copy psum accumulators out so the o slots free early
                ocp = work.tile([65, 1024], F32, tag="ocp", bufs=4)
                nc.vector.tensor_copy(ocp[:, 0:QW], o_A[:])
                nc.vector.tensor_copy(ocp[:, QW:1024], o_B[:])
                norm_q.append((j, p, ocp))
                if len(norm_q) > 1:
                    norm_finish(*norm_q.pop(0))

            proj0()
            # Filler supply per attention window: proj(j+1) is queued during
            # attn(j) and must fully drain before attn(j+1) (in-order tensor
            # queue).  outproj(j) is queued two pairs into attn(j+1) so its
            # ot reads are emitted after the lag-2 normalize muls of (j, 3).
            DRAIN_N = {0: 5, 1: 4, 2: 3, 3: 1}
            for j in range(NJ):
                for p in range(NP):
                    if j + 1 < NJ:
                        pending.append(q_gen(j + 1, p))
                        pending.append(k_gen(j + 1, p))
                        if p == NP - 1:
                            pending.extend([v_gen(j + 1, sub) for sub in range(4)])
                    if j > 0 and p == 1:
                        pending.append(outproj_gen(j - 1))
                    attn_p(j, p, DRAIN_N[j])
                drain_all()
            # tail: finish the last pair, then the last outproj
            norm_recip(*norm_q[-1])
            norm_finish(*norm_q.pop(0))
            pending.append(outproj_gen(3))
            drain_all()

    nc.finalize()
    return nc


def _prep_inputs(x, Wq, Wk, Wv, Wo, bo):
    """Build the 8 per-core input maps (host-side layout prep only)."""
    import ml_dtypes

    FP8 = ml_dtypes.float8_e4m3
    scale = np.float32(1.0 / np.sqrt(np.float32(HD)) * 2.0 ** SQ_EXP)
    kr = np.arange(128, dtype=np.float32)[:, None]
    qc = np.arange(128, dtype=np.float32)[None, :]
    tri = np.where(qc >= kr, np.float32(0.0), np.float32(MASK_VAL))
    mneg = np.tile(tri, (1, 2)).astype(ml_dtypes.bfloat16)

    in_maps = []
    for c in range(NCORES):
        b, g = c // 2, c % 2
        hs = slice(g * 8, (g + 1) * 8)
        xtf = np.ascontiguousarray(x[b].T)
        xt = xtf.astype(FP8)
        xtv = xtf.astype(ml_dtypes.bfloat16)
        wq = np.ascontiguousarray(Wq[hs].reshape(512, D).T * scale).astype(FP8)
        wk = np.ascontiguousarray(Wk[hs].reshape(512, D).T * np.float32(2.0 ** SK_EXP)).astype(FP8)
        wv = np.ascontiguousarray(Wv[hs].reshape(512, D).T).astype(ml_dtypes.bfloat16)
        wo = np.ascontiguousarray(Wo[:, g * 512:(g + 1) * 512].T).astype(ml_dtypes.bfloat16)
        if g == 0:
            bias = np.ascontiguousarray(bo.reshape(8, 128).T)
        else:
            bias = np.zeros((128, 8), np.float32)
        in_maps.append(
            {"xt": xt, "xtv": xtv, "wq": wq, "wk": wk, "wv": wv, "wo": wo,
             "bias": bias, "mask": mneg}
        )
    return in_maps


def _run(inputs, trace=False, trace_cores=None):
    from concourse.bass_utils import run_bass_kernel_spmd

    if "nc" not in _CACHE:
        _CACHE["nc"] = _build_nc()
    nc = _CACHE["nc"]
    in_maps = _prep_inputs(
        inputs["x"], inputs["Wq"], inputs["Wk"], inputs["Wv"], inputs["Wo"], inputs["bo"]
    )
    r = run_bass_kernel_spmd(
        nc, in_maps, list(range(NCORES)), trace=trace, trace_cores=trace_cores
    )
    y = np.empty((B, T, D), np.float32)
    for b in range(B):
        yt = np.asarray(r.results[2 * b]["yt"], dtype=np.float32) + np.asarray(
            r.results[2 * b + 1]["yt"], dtype=np.float32
        )
        y[b] = yt.T
    return y, r


def kernel(**inputs):
    y, _ = _run(inputs, trace=False)
    return y
